# revision 1
# baseline (speedup 1.0000x reference)
"""Biaffine attention kernel for Trainium2, data-parallel over 8 NeuronCores.

Math (per batch b):
    xp = Wf @ x[b] + bf          (128, L)
    yp = Wa @ y[b] + ba          (128, L)
    scores = xp @ yp.T           (128, 128)   contraction over L
    attn = softmax(scores, -1) / sqrt(L)
    out[b] = attn @ (xp + yp)    (128, L)

Distribution: batch dim (32) sharded 4-per-core across 8 cores; weights
replicated. No collectives.

Per-core dataflow:
  - x/y streamed HBM->SBUF in 2 MiB tiles (fp32).
  - projections on TensorE as float32r (FP22 single-pass) matmuls, N=512.
  - PSUM evacuated by ScalarE with fused per-partition bias add, cast to
    fp16 activations (xp16/yp16) kept resident in SBUF for the whole batch.
  - xp16/yp16 transposed 128x128 via TensorE transpose-mode (fp16), PSUM
    evacuated by VectorE into xpT/ypT; scores accumulate over 64 chunks
    into one PSUM bank (fp16 matmuls).
  - softmax rowwise (free dim): DVE max-reduce, ACT exp with fused
    -max bias and sum accumulation, DVE reciprocal; 1/sqrt(L) folded in.
  - out = attnT.T @ xp16 + attnT.T @ yp16 accumulated in PSUM, evacuated
    and DMA'd back as fp32.
"""

import numpy as np

P = 128
L = 8192
B = 32
NCORES = 8
BPC = B // NCORES  # batches per core
SQRT_L = float(np.sqrt(float(L)))

CHUNK = 512  # projection / out matmul free dim
TCH = 128  # transpose chunk
TGRP = 8  # transposes per PSUM bank evacuation
IN_TILE = 4096  # HBM<->SBUF dma tile (2 MiB fp32)


def _patch_tail_drain(tile, mybir, ScopedClock):
    """This container's walrus rejects >1 sync wait on the kernel-tail Drain
    (setupSyncWait: 'Too many sync wait commands'). Spread the tail-drain
    waits across a chain of drains, one wait each."""
    if getattr(tile.TileContext, "_drain_split_patched", False):
        return

    def _split_drain_and_barrier(self, tick_clock, wait_clock):
        nc = self.nc
        drain_inst = nc.sync.drain()
        wait_clock.add_sem_waits(
            drain_inst.ins, ScopedClock({None: tick_clock.global_clock})
        )
        si = drain_inst.ins.sync_info
        if si is not None and si.on_wait is not None and len(si.on_wait) > 1:
            waits = list(si.on_wait)
            si.on_wait = waits[:1]
            for w in waits[1:]:
                extra = nc.sync.drain()
                esi = extra.ins.sync_info
                if esi is None:
                    extra.ins.sync_info = mybir.SyncInfo(on_wait=[w], on_update=[])
                else:
                    ow = list(esi.on_wait) if esi.on_wait else []
                    ow.append(w)
                    esi.on_wait = ow
        nc.all_engine_barrier()
        assert self.sems is not None
        popped = nc._tile_sem_poison_stack.pop()
        assert popped is self._sem_poison
        nc.clear_and_free_semaphores(list(self.sems.allocated().values()))
        nc.all_engine_barrier()

    tile.TileContext._drain_and_barrier = _split_drain_and_barrier
    tile.TileContext._drain_split_patched = True


def _split_excess_waits(nc, mybir, max_waits=1):
    """Walrus in this container rejects instructions carrying more than a
    couple of sync waits ('Too many sync wait commands'). Hoist excess waits
    onto dedicated same-engine NoOps inserted just before the instruction."""
    ctr = 0
    for blk in nc.m.functions[0].blocks:
        new_insts = []
        for inst in blk.instructions:
            si = inst.sync_info
            if si is not None and si.on_wait and len(si.on_wait) > max_waits:
                waits = list(si.on_wait)
                excess, keep = waits[:-max_waits], waits[-max_waits:]
                si.on_wait = keep
                for i in range(0, len(excess), max_waits):
                    ctr += 1
                    nop = mybir.InstNoOp(
                        name=f"I-waitsplit-{ctr}",
                        sync_info=mybir.SyncInfo(
                            on_wait=excess[i : i + max_waits], on_update=[]
                        ),
                        bass_nofuse=True,
                        engine=inst.engine,
                    )
                    nc.register_instruction(nop)
                    new_insts.append(nop)
            new_insts.append(inst)
        blk.instructions = new_insts


def build_nc(bpc=BPC, seq=L, scores_fp32=False):
    import concourse.bass as bass
    import concourse.mybir as mybir
    import concourse.tile as tile
    from concourse.masks import make_identity
    from concourse.vector_clock import ScopedClock

    _patch_tail_drain(tile, mybir, ScopedClock)

    f32 = mybir.dt.float32
    f32r = mybir.dt.float32r
    f16 = mybir.dt.float16
    AF = mybir.ActivationFunctionType
    ALU = mybir.AluOpType
    AX = mybir.AxisListType

    sqrt_l = float(np.sqrt(float(seq)))
    in_tile = min(IN_TILE, seq)
    ntr = seq // TCH  # number of 128-col transpose chunks
    tgrp = min(TGRP, ntr)  # transposes per PSUM bank
    nin = seq // in_tile  # dma tiles per batch
    cpin = in_tile // CHUNK  # matmul chunks per dma tile

    # dtype of the scores operand path (activations, transposes, scores mm)
    sdt = f32 if scores_fp32 else f16

    def r(ap):
        # reduced-precision single-pass view for fp32 matmul operands
        return ap.bitcast(f32r) if ap.dtype == f32 else ap

    nc = bass.Bass("TRN2", target_bir_lowering=False, debug=False)
    x_d = nc.dram_tensor("x", [bpc, P, seq], f32, kind="ExternalInput").ap()
    y_d = nc.dram_tensor("y", [bpc, P, seq], f32, kind="ExternalInput").ap()
    wf_d = nc.dram_tensor("wf", [P, P], f32, kind="ExternalInput").ap()
    bf_d = nc.dram_tensor("bf", [P], f32, kind="ExternalInput").ap()
    wa_d = nc.dram_tensor("wa", [P, P], f32, kind="ExternalInput").ap()
    ba_d = nc.dram_tensor("ba", [P], f32, kind="ExternalInput").ap()
    out_d = nc.dram_tensor("out", [bpc, P, seq], f32, kind="ExternalOutput").ap()

    with tile.TileContext(nc) as tc:
        with (
            tc.tile_pool(name="consts", bufs=1) as consts,
            tc.tile_pool(name="xin", bufs=3) as xin_pool,
            tc.tile_pool(name="acts", bufs=2) as acts_pool,
            tc.tile_pool(name="trs", bufs=1) as tr_pool,
            tc.tile_pool(name="sm", bufs=2) as sm_pool,
            tc.tile_pool(name="outs", bufs=2) as out_pool,
            tc.tile_pool(name="pproj", bufs=3, space="PSUM") as psum_proj,
            tc.tile_pool(name="ptr", bufs=2, space="PSUM") as psum_tr,
            tc.tile_pool(name="psc", bufs=1, space="PSUM") as psum_sc,
            tc.tile_pool(name="pout", bufs=2, space="PSUM") as psum_out,
        ):
            # Issue the first batch's input loads before anything else so the
            # DMA engines are saturated during constant setup (program order
            # drives scheduler priority).
            preloaded = {}
            for h in range(min(2, nin)):
                x_t = xin_pool.tile([P, in_tile], f32r, tag="x_t", name="x_t")
                y_t = xin_pool.tile([P, in_tile], f32r, tag="y_t", name="y_t")
                hs = slice(h * in_tile, (h + 1) * in_tile)
                nc.sync.dma_start(x_t, x_d[0, :, hs].bitcast(f32r))
                nc.sync.dma_start(y_t, y_d[0, :, hs].bitcast(f32r))
                preloaded[h] = (x_t, y_t)

            # ---- constants ----
            wf_nat = consts.tile([P, P], f32)
            nc.sync.dma_start(wf_nat, wf_d)
            wa_nat = consts.tile([P, P], f32)
            nc.sync.dma_start(wa_nat, wa_d)
            id32 = consts.tile([P, P], f32)
            make_identity(nc, id32)
            ids = consts.tile([P, P], sdt)
            make_identity(nc, ids)
            bias_f = consts.tile([P, 1], f32)
            nc.sync.dma_start(bias_f, bf_d.rearrange("(p o) -> p o", o=1))
            bias_a = consts.tile([P, 1], f32)
            nc.sync.dma_start(bias_a, ba_d.rearrange("(p o) -> p o", o=1))

            # WfT/WaT ([in,out] layout) via TensorE transpose; stored as
            # float32r so the BIR verifier accepts them as fp32r matmul
            # operands (producer must write the fp32r dtype).
            wfT = consts.tile([P, P], f32r)
            waT = consts.tile([P, P], f32r)
            for nat, tsp in ((wf_nat, wfT), (wa_nat, waT)):
                pw = psum_proj.tile([P, CHUNK], f32, tag="pp", name="pw")
                nc.tensor.transpose(pw[:, :P], nat, id32)
                nc.vector.tensor_copy(out=tsp, in_=pw[:, :P])

            for b in range(bpc):
                xp16 = acts_pool.tile([P, seq], sdt, tag="xp16", name="xp16")
                yp16 = acts_pool.tile([P, seq], sdt, tag="yp16", name="yp16")

                # ---- phase 1: stream in, project, bias + cast ----
                for h in range(nin):
                    if b == 0 and h in preloaded:
                        x_t, y_t = preloaded[h]
                    else:
                        # float32r tiles (same 4-byte layout as f32); the DRAM
                        # side is bitcast so the DMA is dtype-consistent and
                        # the fp32r matmul sees a properly-typed producer.
                        x_t = xin_pool.tile([P, in_tile], f32r, tag="x_t", name="x_t")
                        y_t = xin_pool.tile([P, in_tile], f32r, tag="y_t", name="y_t")
                        hs = slice(h * in_tile, (h + 1) * in_tile)
                        nc.sync.dma_start(x_t, x_d[b, :, hs].bitcast(f32r))
                        nc.sync.dma_start(y_t, y_d[b, :, hs].bitcast(f32r))
                    for cc in range(cpin):
                        c0 = h * in_tile + cc * CHUNK
                        cs_in = slice(cc * CHUNK, (cc + 1) * CHUNK)
                        cs = slice(c0, c0 + CHUNK)
                        px = psum_proj.tile([P, CHUNK], f32, tag="pp", name="px")
                        nc.tensor.matmul(
                            px, r(wfT[:]), r(x_t[:, cs_in]), start=True, stop=True
                        )
                        nc.scalar.activation(
                            out=xp16[:, cs], in_=px, func=AF.Identity, bias=bias_f
                        )
                        py = psum_proj.tile([P, CHUNK], f32, tag="pp", name="py")
                        nc.tensor.matmul(
                            py, r(waT[:]), r(y_t[:, cs_in]), start=True, stop=True
                        )
                        nc.scalar.activation(
                            out=yp16[:, cs], in_=py, func=AF.Identity, bias=bias_a
                        )

                # ---- phase 2: transpose activations ----
                xpT = tr_pool.tile([P, seq], sdt, tag="xpT", name="xpT")
                ypT = tr_pool.tile([P, seq], sdt, tag="ypT", name="ypT")
                for g in range(ntr // tgrp):
                    ptx = psum_tr.tile([P, tgrp * TCH], sdt, tag="pt", name="ptx")
                    pty = psum_tr.tile([P, tgrp * TCH], sdt, tag="pt", name="pty")
                    for t in range(tgrp):
                        c = g * tgrp + t
                        ts_ = slice(t * TCH, (t + 1) * TCH)
                        cs = slice(c * TCH, (c + 1) * TCH)
                        nc.tensor.transpose(ptx[:, ts_], xp16[:, cs], ids)
                        nc.tensor.transpose(pty[:, ts_], yp16[:, cs], ids)
                    gs = slice(g * tgrp * TCH, (g + 1) * tgrp * TCH)
                    nc.vector.tensor_copy(out=xpT[:, gs], in_=ptx)
                    nc.vector.tensor_copy(out=ypT[:, gs], in_=pty)

                # ---- phase 3: scores (accumulate over seq chunks) ----
                ps = psum_sc.tile([P, P], f32, tag="ps", name="ps")
                for c in range(ntr):
                    cs = slice(c * TCH, (c + 1) * TCH)
                    nc.tensor.matmul(
                        ps,
                        xpT[:, cs],
                        ypT[:, cs],
                        start=(c == 0),
                        stop=(c == ntr - 1),
                    )

                # ---- phase 4: softmax (rowwise over free dim) ----
                negmx = sm_pool.tile([P, 1], f32, tag="negmx", name="negmx")
                nc.vector.tensor_reduce(
                    out=negmx, in_=ps, axis=AX.X, op=ALU.max, negate=True
                )
                e = sm_pool.tile([P, P], f32, tag="e", name="e")
                se = sm_pool.tile([P, 1], f32, tag="se", name="se")
                nc.scalar.activation(
                    out=e, in_=ps, func=AF.Exp, bias=negmx, scale=1.0, accum_out=se
                )
                sse = sm_pool.tile([P, 1], f32, tag="sse", name="sse")
                nc.vector.tensor_scalar_mul(sse, se, sqrt_l)
                rcp = sm_pool.tile([P, 1], f32, tag="rcp", name="rcp")
                nc.vector.reciprocal(rcp, sse)
                attn = sm_pool.tile([P, P], sdt, tag="attn", name="attn")
                nc.vector.tensor_scalar_mul(attn, e, rcp)
                pat = psum_tr.tile([P, tgrp * TCH], sdt, tag="pt", name="pat")
                nc.tensor.transpose(pat[:, :P], attn, ids)
                attnT = sm_pool.tile([P, P], sdt, tag="attnT", name="attnT")
                nc.vector.tensor_copy(out=attnT, in_=pat[:, :P])

                # ---- phase 5: out = attnT.T @ (xp + yp), stream back ----
                out_tile = min(1024, seq)
                nout = seq // out_tile
                cpo = out_tile // CHUNK
                for h in range(nout):
                    ot = out_pool.tile([P, out_tile], f32, tag="ot", name="ot")
                    for cc in range(cpo):
                        c0 = h * out_tile + cc * CHUNK
                        cs = slice(c0, c0 + CHUNK)
                        po = psum_out.tile([P, CHUNK], f32, tag="po", name="po")
                        nc.tensor.matmul(
                            po, attnT[:], xp16[:, cs], start=True, stop=False
                        )
                        nc.tensor.matmul(
                            po, attnT[:], yp16[:, cs], start=False, stop=True
                        )
                        nc.any.tensor_copy(
                            out=ot[:, cc * CHUNK : (cc + 1) * CHUNK], in_=po
                        )
                    hs = slice(h * out_tile, (h + 1) * out_tile)
                    # stores issue from the ACT HWDGE ring so they don't
                    # share the SP ring with input loads
                    nc.scalar.dma_start(out_d[b, :, hs], ot)

    _split_excess_waits(nc, mybir, max_waits=1)
    return nc


_nc_cache = {}


def _get_nc():
    key = (BPC, L)
    if key not in _nc_cache:
        _nc_cache[key] = build_nc(BPC, L)
    return _nc_cache[key]


def kernel(x, y, Wf, bf, Wa, ba):
    from concourse.bass_utils import run_bass_kernel_spmd

    x = np.asarray(x, dtype=np.float32)
    y = np.asarray(y, dtype=np.float32)
    Wf = np.ascontiguousarray(np.asarray(Wf, dtype=np.float32))
    bf = np.ascontiguousarray(np.asarray(bf, dtype=np.float32))
    Wa = np.ascontiguousarray(np.asarray(Wa, dtype=np.float32))
    ba = np.ascontiguousarray(np.asarray(ba, dtype=np.float32))

    nc = _get_nc()
    in_maps = []
    for c in range(NCORES):
        sl = slice(c * BPC, (c + 1) * BPC)
        in_maps.append(
            {
                "x": np.ascontiguousarray(x[sl]),
                "y": np.ascontiguousarray(y[sl]),
                "wf": Wf,
                "bf": bf,
                "wa": Wa,
                "ba": ba,
            }
        )
    res = run_bass_kernel_spmd(nc, in_maps, core_ids=list(range(NCORES)))
    out = np.concatenate([r["out"] for r in res.results], axis=0)
    return np.ascontiguousarray(out.astype(np.float32))


if __name__ == "__main__":
    rng = np.random.default_rng(0)
    inputs = {
        "x": rng.standard_normal((B, P, L), dtype=np.float32),
        "y": rng.standard_normal((B, P, L), dtype=np.float32),
        "Wf": (rng.standard_normal((P, P)) / np.sqrt(P)).astype(np.float32),
        "bf": (rng.standard_normal(P) * 0.02).astype(np.float32),
        "Wa": (rng.standard_normal((P, P)) / np.sqrt(P)).astype(np.float32),
        "ba": (rng.standard_normal(P) * 0.02).astype(np.float32),
    }
    o = kernel(**inputs)
    print(o.shape, o.dtype)



# revision 6
# speedup vs baseline: 1.0798x; 1.0798x over previous
"""Biaffine attention kernel for Trainium2, data-parallel over 8 NeuronCores.

Math (per batch b):
    xp = Wf @ x[b] + bf          (128, L)
    yp = Wa @ y[b] + ba          (128, L)
    scores = xp @ yp.T           (128, 128)   contraction over L
    attn = softmax(scores, -1) / sqrt(L)
    out[b] = attn @ (xp + yp)    (128, L)

Distribution: batch dim (32) sharded 4-per-core across 8 cores; weights
replicated. No collectives.

v2 dataflow ("route E", fp16 I/O):
  - x/y are cast to fp16 on the host; HBM reads drop to 16 MiB/core and the
    output is written fp16 (8 MiB/core) and cast back on the host. DMA floor
    ~70us/core instead of ~140us.
  - Projections are computed TRANSPOSED: per 128-col chunk c,
    xpT_c[l,o] = matmul(lhsT=x_c[f,l], rhs=WfT[f,o]) so no separate
    activation-transpose pass is needed for the scores contraction.
  - zT = xpT + ypT (DVE add); z natural recovered with one PE transpose per
    chunk (8 chunks batched per fp16 PSUM bank), evacuated by ScalarE with
    the fused (bf+ba) per-partition bias.
  - Biases enter scores exactly via a rank-2 correction computed HOST-side:
    scores += bf (x) (v0 + L*ba) + u0 (x) ba, appended as a final k=2 matmul
    into the scores PSUM accumulation.
  - Evacuations are spread across DVE (xpT, zT add), Pool (ypT), ScalarE (z).
  - Batches are software-pipelined: out(b-1) is emitted between phase1(b)
    and softmax(b) so the PE never waits on the softmax chain.
"""

import numpy as np

P = 128
L = 8192
B = 32
NCORES = 8
BPC = B // NCORES  # batches per core
SQRT_L = float(np.sqrt(float(L)))

IN_TILE = 4096  # HBM->SBUF dma tile (1 MiB fp16)
OUT_TILE = 512  # out matmul free dim / store tile
ZGRP = 8  # z transpose-backs per fp16 PSUM bank
SCORES_LAG = 2  # groups (of 2 chunks) between proj-evac and scores mm
ZBACK_LAG = 3  # groups between zT add and transpose-back


def _patch_tail_drain(tile, mybir, ScopedClock):
    """This container's walrus rejects >1 sync wait on the kernel-tail Drain
    (setupSyncWait: 'Too many sync wait commands'). Spread the tail-drain
    waits across a chain of drains, one wait each."""
    if getattr(tile.TileContext, "_drain_split_patched", False):
        return

    def _split_drain_and_barrier(self, tick_clock, wait_clock):
        nc = self.nc
        drain_inst = nc.sync.drain()
        wait_clock.add_sem_waits(
            drain_inst.ins, ScopedClock({None: tick_clock.global_clock})
        )
        si = drain_inst.ins.sync_info
        if si is not None and si.on_wait is not None and len(si.on_wait) > 1:
            waits = list(si.on_wait)
            si.on_wait = waits[:1]
            for w in waits[1:]:
                extra = nc.sync.drain()
                esi = extra.ins.sync_info
                if esi is None:
                    extra.ins.sync_info = mybir.SyncInfo(on_wait=[w], on_update=[])
                else:
                    ow = list(esi.on_wait) if esi.on_wait else []
                    ow.append(w)
                    esi.on_wait = ow
        nc.all_engine_barrier()
        assert self.sems is not None
        popped = nc._tile_sem_poison_stack.pop()
        assert popped is self._sem_poison
        nc.clear_and_free_semaphores(list(self.sems.allocated().values()))
        nc.all_engine_barrier()

    tile.TileContext._drain_and_barrier = _split_drain_and_barrier
    tile.TileContext._drain_split_patched = True


def _split_excess_waits(nc, mybir, max_waits=1):
    """Walrus in this container rejects instructions carrying more than a
    couple of sync waits ('Too many sync wait commands'). Hoist excess waits
    onto dedicated same-engine NoOps inserted just before the instruction."""
    ctr = 0
    for blk in nc.m.functions[0].blocks:
        new_insts = []
        for inst in blk.instructions:
            si = inst.sync_info
            if si is not None and si.on_wait and len(si.on_wait) > max_waits:
                waits = list(si.on_wait)
                excess, keep = waits[:-max_waits], waits[-max_waits:]
                si.on_wait = keep
                for i in range(0, len(excess), max_waits):
                    ctr += 1
                    nop = mybir.InstNoOp(
                        name=f"I-waitsplit-{ctr}",
                        sync_info=mybir.SyncInfo(
                            on_wait=excess[i : i + max_waits], on_update=[]
                        ),
                        bass_nofuse=True,
                        engine=inst.engine,
                    )
                    nc.register_instruction(nop)
                    new_insts.append(nop)
            new_insts.append(inst)
        blk.instructions = new_insts


def build_nc(bpc=BPC, seq=L):
    import concourse.bass as bass
    import concourse.mybir as mybir
    import concourse.tile as tile
    from concourse.masks import make_identity
    from concourse.vector_clock import ScopedClock

    _patch_tail_drain(tile, mybir, ScopedClock)

    f32 = mybir.dt.float32
    f16 = mybir.dt.float16
    AF = mybir.ActivationFunctionType
    ALU = mybir.AluOpType
    AX = mybir.AxisListType

    sqrt_l = float(np.sqrt(float(seq)))
    nchunk = seq // P  # 128-col proj chunks per batch
    ngrp = nchunk // 2  # 2 chunks per PSUM AB bank
    tiles_per_b = seq // IN_TILE
    grp_per_tile = IN_TILE // 256
    nout = seq // OUT_TILE

    nc = bass.Bass("TRN2", target_bir_lowering=False, debug=False)
    x_d = nc.dram_tensor("x", [bpc, P, seq], f16, kind="ExternalInput").ap()
    y_d = nc.dram_tensor("y", [bpc, P, seq], f16, kind="ExternalInput").ap()
    wft_d = nc.dram_tensor("wft", [P, P], f16, kind="ExternalInput").ap()
    wat_d = nc.dram_tensor("wat", [P, P], f16, kind="ExternalInput").ap()
    bzv_d = nc.dram_tensor("bzv", [P], f32, kind="ExternalInput").ap()
    l2_d = nc.dram_tensor("l2", [bpc, 2, P], f16, kind="ExternalInput").ap()
    r2_d = nc.dram_tensor("r2", [bpc, 2, P], f16, kind="ExternalInput").ap()
    out_d = nc.dram_tensor("out", [bpc, P, seq], f16, kind="ExternalOutput").ap()

    with tile.TileContext(nc) as tc:
        with (
            tc.tile_pool(name="consts", bufs=1) as consts,
            tc.tile_pool(name="xin", bufs=4) as xin_pool,
            tc.tile_pool(name="acts", bufs=1) as acts_pool,
            tc.tile_pool(name="zp", bufs=2) as z_pool,
            tc.tile_pool(name="sm", bufs=2) as sm_pool,
            tc.tile_pool(name="outs", bufs=4) as out_pool,
            tc.tile_pool(name="pab", bufs=3, space="PSUM") as psum_ab,
            tc.tile_pool(name="pz", bufs=2, space="PSUM") as psum_z,
            tc.tile_pool(name="psc", bufs=1, space="PSUM") as psum_sc,
            tc.tile_pool(name="pout", bufs=2, space="PSUM") as psum_out,
        ):
            # Issue the first batch's input loads before anything else so the
            # DMA engines are busy during constant setup (program order
            # drives scheduler priority).
            preloaded = {}
            for h in range(min(2, tiles_per_b)):
                x_t = xin_pool.tile([P, IN_TILE], f16, tag="x_t", name="x_t")
                y_t = xin_pool.tile([P, IN_TILE], f16, tag="y_t", name="y_t")
                hs = slice(h * IN_TILE, (h + 1) * IN_TILE)
                nc.sync.dma_start(x_t, x_d[0, :, hs])
                nc.sync.dma_start(y_t, y_d[0, :, hs])
                preloaded[h] = (x_t, y_t)

            # ---- constants ----
            wf_sb = consts.tile([P, P], f16)
            nc.sync.dma_start(wf_sb, wft_d)
            wa_sb = consts.tile([P, P], f16)
            nc.sync.dma_start(wa_sb, wat_d)
            ids = consts.tile([P, P], f16)
            make_identity(nc, ids)
            bzv_sb = consts.tile([P, 1], f32)
            nc.sync.dma_start(bzv_sb, bzv_d.rearrange("(p o) -> p o", o=1))
            l2_sb = consts.tile([2, bpc * P], f16)
            r2_sb = consts.tile([2, bpc * P], f16)
            for bb in range(bpc):
                bs = slice(bb * P, (bb + 1) * P)
                nc.sync.dma_start(l2_sb[:, bs], l2_d[bb])
                nc.sync.dma_start(r2_sb[:, bs], r2_d[bb])

            ctx = {}  # per-batch tiles needed by the lagged out phase

            def emit_phase1(b):
                xpT = acts_pool.tile([P, seq], f16, tag="xpT", name="xpT")
                ypT = acts_pool.tile([P, seq], f16, tag="ypT", name="ypT")
                zT = acts_pool.tile([P, seq], f16, tag="zT", name="zT")
                z = z_pool.tile([P, seq], f16, tag="z", name="z")
                ps_t = psum_sc.tile([P, P], f32, tag="ps", name="ps_t")
                x_t = y_t = None
                pz_t = None

                def emit_scores(g):
                    for j in range(2):
                        c = 2 * g + j
                        cs = slice(c * P, (c + 1) * P)
                        nc.tensor.matmul(
                            ps_t,
                            xpT[:, cs],
                            ypT[:, cs],
                            start=(c == 0),
                            stop=False,
                        )

                def emit_zback(g):
                    nonlocal pz_t
                    for j in range(2):
                        c = 2 * g + j
                        zi = c % ZGRP
                        if zi == 0:
                            pz_t = psum_z.tile(
                                [P, ZGRP * P], f16, tag="pz", name="pz_t"
                            )
                        cs = slice(c * P, (c + 1) * P)
                        nc.tensor.transpose(
                            pz_t[:, zi * P : (zi + 1) * P], zT[:, cs], ids
                        )
                        if zi == ZGRP - 1:
                            gs = slice((c - ZGRP + 1) * P, (c + 1) * P)
                            nc.scalar.activation(
                                out=z[:, gs],
                                in_=pz_t,
                                func=AF.Identity,
                                bias=bzv_sb,
                            )

                for g in range(ngrp):
                    if g % grp_per_tile == 0:
                        h = g // grp_per_tile
                        if b == 0 and h in preloaded:
                            x_t, y_t = preloaded[h]
                        else:
                            x_t = xin_pool.tile([P, IN_TILE], f16, tag="x_t", name="x_t")
                            y_t = xin_pool.tile([P, IN_TILE], f16, tag="y_t", name="y_t")
                            hs = slice(h * IN_TILE, (h + 1) * IN_TILE)
                            nc.sync.dma_start(x_t, x_d[b, :, hs])
                            nc.sync.dma_start(y_t, y_d[b, :, hs])
                    pab_t = psum_ab.tile([P, 4 * P], f32, tag="pp", name="pab_t")
                    # A(j=0), A(j=1), then B(j=0), B(j=1): the DVE evac of the
                    # A half can start two matmuls earlier than the Pool evac.
                    for j in range(2):
                        lc = (2 * g + j) * P - (g // grp_per_tile) * IN_TILE
                        nc.tensor.matmul(
                            pab_t[:, j * P : (j + 1) * P],
                            x_t[:, lc : lc + P],
                            wf_sb,
                            start=True,
                            stop=True,
                        )
                    for j in range(2):
                        lc = (2 * g + j) * P - (g // grp_per_tile) * IN_TILE
                        nc.tensor.matmul(
                            pab_t[:, (2 + j) * P : (3 + j) * P],
                            y_t[:, lc : lc + P],
                            wa_sb,
                            start=True,
                            stop=True,
                        )
                    gs = slice(2 * g * P, (2 * g + 2) * P)
                    # Pool cannot read PSUM on TRN2: DVE takes both PSUM
                    # evacs, Pool does the SBUF-only zT add.
                    nc.vector.tensor_copy(out=xpT[:, gs], in_=pab_t[:, 0 : 2 * P])
                    nc.vector.tensor_copy(out=ypT[:, gs], in_=pab_t[:, 2 * P : 4 * P])
                    nc.gpsimd.tensor_add(zT[:, gs], xpT[:, gs], ypT[:, gs])
                    if g >= SCORES_LAG:
                        emit_scores(g - SCORES_LAG)
                    if g >= ZBACK_LAG:
                        emit_zback(g - ZBACK_LAG)
                for g in range(ngrp - SCORES_LAG, ngrp):
                    emit_scores(g)
                for g in range(ngrp - ZBACK_LAG, ngrp):
                    emit_zback(g)
                # rank-2 bias correction, final accumulation into scores
                bs = slice(b * P, (b + 1) * P)
                nc.tensor.matmul(
                    ps_t, l2_sb[:, bs], r2_sb[:, bs], start=False, stop=True
                )
                ctx[b] = {"z": z, "ps": ps_t}

            def emit_softmax(b):
                ps_t = ctx[b]["ps"]
                negmx = sm_pool.tile([P, 1], f32, tag="negmx", name="negmx")
                nc.vector.tensor_reduce(
                    out=negmx, in_=ps_t, axis=AX.X, op=ALU.max, negate=True
                )
                e = sm_pool.tile([P, P], f32, tag="e", name="e")
                se = sm_pool.tile([P, 1], f32, tag="se", name="se")
                nc.scalar.activation(
                    out=e, in_=ps_t, func=AF.Exp, bias=negmx, scale=1.0, accum_out=se
                )
                sse = sm_pool.tile([P, 1], f32, tag="sse", name="sse")
                nc.vector.tensor_scalar_mul(sse, se, sqrt_l)
                rcp = sm_pool.tile([P, 1], f32, tag="rcp", name="rcp")
                nc.vector.reciprocal(rcp, sse)
                attn = sm_pool.tile([P, P], f16, tag="attn", name="attn")
                nc.vector.tensor_scalar_mul(attn, e, rcp)
                pat = psum_sc.tile([P, P], f16, tag="ps", name="pat")
                nc.tensor.transpose(pat, attn, ids)
                attnT = sm_pool.tile([P, P], f16, tag="attnT", name="attnT")
                nc.vector.tensor_copy(out=attnT, in_=pat)
                ctx[b]["attnT"] = attnT

            def emit_out(b):
                attnT = ctx[b]["attnT"]
                z = ctx[b]["z"]
                for oc in range(nout):
                    os_ = slice(oc * OUT_TILE, (oc + 1) * OUT_TILE)
                    po_t = psum_out.tile([P, OUT_TILE], f32, tag="po", name="po_t")
                    nc.tensor.matmul(
                        po_t, attnT, z[:, os_], start=True, stop=True
                    )
                    ot = out_pool.tile([P, OUT_TILE], f16, tag="ot", name="ot")
                    cut = 320  # DVE is ~1.8x faster than ACT per col
                    nc.vector.tensor_copy(out=ot[:, :cut], in_=po_t[:, :cut])
                    nc.scalar.activation(
                        out=ot[:, cut:], in_=po_t[:, cut:], func=AF.Identity
                    )
                    nc.scalar.dma_start(out_d[b, :, os_], ot)
                del ctx[b]

            for b in range(bpc):
                emit_phase1(b)
                if b > 0:
                    emit_out(b - 1)
                emit_softmax(b)
            emit_out(bpc - 1)

    _split_excess_waits(nc, mybir, max_waits=1)
    return nc


_nc_cache = {}


def _get_nc():
    key = (BPC, L)
    if key not in _nc_cache:
        _nc_cache[key] = build_nc(BPC, L)
    return _nc_cache[key]


def _prep_host(x, y, Wf, bf, Wa, ba):
    """Cast inputs to fp16 and compute the rank-2 scores bias correction."""
    x16 = np.ascontiguousarray(x.astype(np.float16))
    y16 = np.ascontiguousarray(y.astype(np.float16))
    wf16 = Wf.astype(np.float16)
    wa16 = Wa.astype(np.float16)
    # rowsums of the fp16-projected activations (without bias):
    # u0 = Wf @ sum_l x, v0 = Wa @ sum_l y  (fp32 accumulation)
    sx = x16.astype(np.float32).sum(axis=-1)  # (B, 128)
    sy = y16.astype(np.float32).sum(axis=-1)
    u0 = sx @ wf16.astype(np.float32).T  # (B, 128)
    v0 = sy @ wa16.astype(np.float32).T
    nb = x.shape[0]
    l2 = np.empty((nb, 2, P), np.float16)
    r2 = np.empty((nb, 2, P), np.float16)
    l2[:, 0, :] = bf[None, :]
    l2[:, 1, :] = u0
    r2[:, 0, :] = v0 + float(L) * ba[None, :]
    r2[:, 1, :] = ba[None, :]
    wft = np.ascontiguousarray(wf16.T)
    wat = np.ascontiguousarray(wa16.T)
    bzv = (bf + ba).astype(np.float32)
    return x16, y16, wft, wat, bzv, l2, r2


def make_in_maps(x, y, Wf, bf, Wa, ba):
    x = np.asarray(x, dtype=np.float32)
    y = np.asarray(y, dtype=np.float32)
    Wf = np.asarray(Wf, dtype=np.float32)
    bf = np.asarray(bf, dtype=np.float32)
    Wa = np.asarray(Wa, dtype=np.float32)
    ba = np.asarray(ba, dtype=np.float32)
    x16, y16, wft, wat, bzv, l2, r2 = _prep_host(x, y, Wf, bf, Wa, ba)
    in_maps = []
    for c in range(NCORES):
        sl = slice(c * BPC, (c + 1) * BPC)
        in_maps.append(
            {
                "x": np.ascontiguousarray(x16[sl]),
                "y": np.ascontiguousarray(y16[sl]),
                "wft": wft,
                "wat": wat,
                "bzv": bzv,
                "l2": np.ascontiguousarray(l2[sl]),
                "r2": np.ascontiguousarray(r2[sl]),
            }
        )
    return in_maps


def kernel(x, y, Wf, bf, Wa, ba):
    from concourse.bass_utils import run_bass_kernel_spmd

    in_maps = make_in_maps(x, y, Wf, bf, Wa, ba)
    nc = _get_nc()
    res = run_bass_kernel_spmd(nc, in_maps, core_ids=list(range(NCORES)))
    out = np.concatenate([r["out"] for r in res.results], axis=0)
    return np.ascontiguousarray(out.astype(np.float32))


if __name__ == "__main__":
    rng = np.random.default_rng(0)
    inputs = {
        "x": rng.standard_normal((B, P, L), dtype=np.float32),
        "y": rng.standard_normal((B, P, L), dtype=np.float32),
        "Wf": (rng.standard_normal((P, P)) / np.sqrt(P)).astype(np.float32),
        "bf": (rng.standard_normal(P) * 0.02).astype(np.float32),
        "Wa": (rng.standard_normal((P, P)) / np.sqrt(P)).astype(np.float32),
        "ba": (rng.standard_normal(P) * 0.02).astype(np.float32),
    }
    o = kernel(**inputs)
    print(o.shape, o.dtype)


# revision 9
# speedup vs baseline: 1.3886x; 1.2860x over previous
"""Biaffine attention kernel for Trainium2, data-parallel over 8 NeuronCores.

Math (per batch b):
    xp = Wf @ x[b] + bf          (128, L)
    yp = Wa @ y[b] + ba          (128, L)
    scores = xp @ yp.T           (128, 128)   contraction over L
    attn = softmax(scores, -1) / sqrt(L)
    out[b] = attn @ (xp + yp)    (128, L)

Distribution: batch dim (32) sharded 4-per-core across 8 cores; weights
replicated. No collectives.

v3 dataflow ("route E2", fp16 I/O):
  - x/y cast to fp16 on the host (HBM reads 16 MiB/core), output written fp16
    (8 MiB/core) and cast back on the host. DMA floor ~70us/core.
  - Projections computed TRANSPOSED per 128-col chunk:
    xpT_c[l,o] = matmul(lhsT=x_c[f,l], rhs=WfT[f,o]); the PSUM->SBUF CAST
    evacuations (DVE for xpT, ScalarE for ypT, 512 cols each) directly
    produce the scores operands - no separate transpose pass.
  - z natural is rebuilt with PAIRS of fp16 PE transposes accumulating in
    PSUM (z_c = T(xpT_c) + T(ypT_c)); fp16-PSUM evacs are ~2x cheaper than
    fp32 CASTs and carry the fused (bf+ba) bias (ACT bias / DVE
    tensor_scalar_add, alternating).
  - Biases enter scores exactly via a HOST-computed rank-2 correction
    appended as a final k=2 matmul into the scores PSUM accumulation.
  - out = attnT.T @ z, one 512-col matmul per chunk; evacs alternate
    DVE/ScalarE; stores ride the ACT HWDGE ring.
  - Batches software-pipelined: out(b-1) emitted between phase1(b) and
    softmax(b) so the PE never waits on the softmax chain.
"""

import numpy as np

P = 128
L = 8192
B = 32
NCORES = 8
BPC = B // NCORES  # batches per core
SQRT_L = float(np.sqrt(float(L)))

IN_TILE = 4096  # HBM->SBUF dma tile (1 MiB fp16)
OUT_TILE = 512  # out matmul free dim / store tile
GRP = 4  # proj chunks per PSUM bank (512 cols)
SCORES_LAG = 1  # groups between proj-evac and scores mms
ZBACK_LAG = 2  # groups between proj-evac and z transpose-backs


def _patch_tail_drain(tile, mybir, ScopedClock):
    """This container's walrus rejects >1 sync wait on the kernel-tail Drain
    (setupSyncWait: 'Too many sync wait commands'). Spread the tail-drain
    waits across a chain of drains, one wait each."""
    if getattr(tile.TileContext, "_drain_split_patched", False):
        return

    def _split_drain_and_barrier(self, tick_clock, wait_clock):
        nc = self.nc
        drain_inst = nc.sync.drain()
        wait_clock.add_sem_waits(
            drain_inst.ins, ScopedClock({None: tick_clock.global_clock})
        )
        si = drain_inst.ins.sync_info
        if si is not None and si.on_wait is not None and len(si.on_wait) > 1:
            waits = list(si.on_wait)
            si.on_wait = waits[:1]
            for w in waits[1:]:
                extra = nc.sync.drain()
                esi = extra.ins.sync_info
                if esi is None:
                    extra.ins.sync_info = mybir.SyncInfo(on_wait=[w], on_update=[])
                else:
                    ow = list(esi.on_wait) if esi.on_wait else []
                    ow.append(w)
                    esi.on_wait = ow
        nc.all_engine_barrier()
        assert self.sems is not None
        popped = nc._tile_sem_poison_stack.pop()
        assert popped is self._sem_poison
        nc.clear_and_free_semaphores(list(self.sems.allocated().values()))
        nc.all_engine_barrier()

    tile.TileContext._drain_and_barrier = _split_drain_and_barrier
    tile.TileContext._drain_split_patched = True


def _split_excess_waits(nc, mybir, max_waits=1):
    """Walrus in this container rejects instructions carrying more than a
    couple of sync waits ('Too many sync wait commands'). Hoist excess waits
    onto dedicated same-engine NoOps inserted just before the instruction."""
    ctr = 0
    for blk in nc.m.functions[0].blocks:
        new_insts = []
        for inst in blk.instructions:
            si = inst.sync_info
            if si is not None and si.on_wait and len(si.on_wait) > max_waits:
                waits = list(si.on_wait)
                excess, keep = waits[:-max_waits], waits[-max_waits:]
                si.on_wait = keep
                for i in range(0, len(excess), max_waits):
                    ctr += 1
                    nop = mybir.InstNoOp(
                        name=f"I-waitsplit-{ctr}",
                        sync_info=mybir.SyncInfo(
                            on_wait=excess[i : i + max_waits], on_update=[]
                        ),
                        bass_nofuse=True,
                        engine=inst.engine,
                    )
                    nc.register_instruction(nop)
                    new_insts.append(nop)
            new_insts.append(inst)
        blk.instructions = new_insts


def build_nc(bpc=BPC, seq=L):
    import concourse.bass as bass
    import concourse.mybir as mybir
    import concourse.tile as tile
    from concourse.masks import make_identity
    from concourse.vector_clock import ScopedClock

    _patch_tail_drain(tile, mybir, ScopedClock)

    f32 = mybir.dt.float32
    f16 = mybir.dt.float16
    AF = mybir.ActivationFunctionType
    ALU = mybir.AluOpType
    AX = mybir.AxisListType

    sqrt_l = float(np.sqrt(float(seq)))
    nchunk = seq // P  # 128-col proj chunks per batch (64)
    ngrp = nchunk // GRP  # 4-chunk groups (16)
    grp_per_tile = IN_TILE // (GRP * P)  # groups per input dma tile (8)
    nout = seq // OUT_TILE
    GW = GRP * P  # group width in cols (512)

    nc = bass.Bass("TRN2", target_bir_lowering=False, debug=False)
    x_d = nc.dram_tensor("x", [bpc, P, seq], f16, kind="ExternalInput").ap()
    y_d = nc.dram_tensor("y", [bpc, P, seq], f16, kind="ExternalInput").ap()
    wft_d = nc.dram_tensor("wft", [P, P], f16, kind="ExternalInput").ap()
    wat_d = nc.dram_tensor("wat", [P, P], f16, kind="ExternalInput").ap()
    bzv_d = nc.dram_tensor("bzv", [P], f32, kind="ExternalInput").ap()
    l2_d = nc.dram_tensor("l2", [bpc, 2, P], f16, kind="ExternalInput").ap()
    r2_d = nc.dram_tensor("r2", [bpc, 2, P], f16, kind="ExternalInput").ap()
    out_d = nc.dram_tensor("out", [bpc, P, seq], f16, kind="ExternalOutput").ap()

    with tile.TileContext(nc) as tc:
        with (
            tc.tile_pool(name="consts", bufs=1) as consts,
            tc.tile_pool(name="xin", bufs=4) as xin_pool,
            tc.tile_pool(name="acts", bufs=1) as acts_pool,
            tc.tile_pool(name="zp", bufs=2) as z_pool,
            tc.tile_pool(name="sm", bufs=2) as sm_pool,
            tc.tile_pool(name="outs", bufs=4) as out_pool,
            tc.tile_pool(name="pp", bufs=3, space="PSUM") as psum_p,
            tc.tile_pool(name="pz", bufs=2, space="PSUM") as psum_z,
            tc.tile_pool(name="psc", bufs=1, space="PSUM") as psum_sc,
            tc.tile_pool(name="pout", bufs=2, space="PSUM") as psum_out,
        ):
            # First batch's input loads go first so the DMA engines are busy
            # during constant setup (program order drives priority).
            preloaded = {}
            for h in range(min(2, seq // IN_TILE)):
                x_t = xin_pool.tile([P, IN_TILE], f16, tag="x_t", name="x_t")
                y_t = xin_pool.tile([P, IN_TILE], f16, tag="y_t", name="y_t")
                hs = slice(h * IN_TILE, (h + 1) * IN_TILE)
                nc.sync.dma_start(x_t, x_d[0, :, hs])
                nc.sync.dma_start(y_t, y_d[0, :, hs])
                preloaded[h] = (x_t, y_t)

            # ---- constants ----
            wf_sb = consts.tile([P, P], f16)
            nc.sync.dma_start(wf_sb, wft_d)
            wa_sb = consts.tile([P, P], f16)
            nc.sync.dma_start(wa_sb, wat_d)
            ids = consts.tile([P, P], f16)
            make_identity(nc, ids)
            bzv_sb = consts.tile([P, 1], f32)
            nc.sync.dma_start(bzv_sb, bzv_d.rearrange("(p o) -> p o", o=1))
            l2_sb = consts.tile([2, bpc * P], f16)
            r2_sb = consts.tile([2, bpc * P], f16)
            for bb in range(bpc):
                bs = slice(bb * P, (bb + 1) * P)
                nc.sync.dma_start(l2_sb[:, bs], l2_d[bb])
                nc.sync.dma_start(r2_sb[:, bs], r2_d[bb])

            ctx = {}  # per-batch tiles needed by the lagged out phase

            def emit_phase1(b):
                xpT = acts_pool.tile([P, seq], f16, tag="xpT", name="xpT")
                ypT = acts_pool.tile([P, seq], f16, tag="ypT", name="ypT")
                z = z_pool.tile([P, seq], f16, tag="z", name="z")
                ps_t = psum_sc.tile([P, P], f32, tag="ps", name="ps_t")
                x_t = y_t = None
                pz_t = None

                def emit_scores(g):
                    for j in range(GRP):
                        c = GRP * g + j
                        cs = slice(c * P, (c + 1) * P)
                        nc.tensor.matmul(
                            ps_t, xpT[:, cs], ypT[:, cs],
                            start=(c == 0), stop=False,
                        )

                pz_banks = {}

                def emit_zback(g):
                    # two fp16 transposes accumulate z_c = T(xpT_c)+T(ypT_c)
                    bk = g // 2
                    if bk not in pz_banks:
                        pz_banks[bk] = psum_z.tile(
                            [P, 2 * GW], f16, tag="pz", name="pz_t"
                        )
                    pzt = pz_banks[bk]
                    for j in range(GRP):
                        c = GRP * g + j
                        cs = slice(c * P, (c + 1) * P)
                        zi = c % (2 * GRP)
                        zs = slice(zi * P, (zi + 1) * P)
                        nc.tensor.matmul(
                            pzt[:, zs], xpT[:, cs], ids,
                            is_transpose=True, start=True, stop=False,
                        )
                        nc.tensor.matmul(
                            pzt[:, zs], ypT[:, cs], ids,
                            is_transpose=True, start=False, stop=True,
                        )

                def emit_zevac(bk):
                    # evac the completed 8-chunk fp16 bank (groups 2bk, 2bk+1)
                    gs = slice(2 * bk * GW, (2 * bk + 2) * GW)
                    pzt = pz_banks.pop(bk)
                    if bk % 2 == 0:
                        nc.scalar.activation(
                            out=z[:, gs], in_=pzt, func=AF.Identity, bias=bzv_sb
                        )
                    else:
                        nc.vector.tensor_scalar_add(z[:, gs], pzt, bzv_sb)

                for g in range(ngrp):
                    if g % grp_per_tile == 0:
                        h = g // grp_per_tile
                        if b == 0 and h in preloaded:
                            x_t, y_t = preloaded[h]
                        else:
                            x_t = xin_pool.tile([P, IN_TILE], f16, tag="x_t", name="x_t")
                            y_t = xin_pool.tile([P, IN_TILE], f16, tag="y_t", name="y_t")
                            hs = slice(h * IN_TILE, (h + 1) * IN_TILE)
                            nc.sync.dma_start(x_t, x_d[b, :, hs])
                            nc.sync.dma_start(y_t, y_d[b, :, hs])
                    ppA = psum_p.tile([P, GW], f32, tag="pp", name="ppA")
                    for j in range(GRP):
                        lc = (GRP * g + j) * P - (g // grp_per_tile) * IN_TILE
                        nc.tensor.matmul(
                            ppA[:, j * P : (j + 1) * P],
                            x_t[:, lc : lc + P], wf_sb,
                            start=True, stop=True,
                        )
                    ppB = psum_p.tile([P, GW], f32, tag="pp", name="ppB")
                    for j in range(GRP):
                        lc = (GRP * g + j) * P - (g // grp_per_tile) * IN_TILE
                        nc.tensor.matmul(
                            ppB[:, j * P : (j + 1) * P],
                            y_t[:, lc : lc + P], wa_sb,
                            start=True, stop=True,
                        )
                    gs = slice(GRP * g * P, GRP * (g + 1) * P)
                    nc.vector.tensor_copy(out=xpT[:, gs], in_=ppA)
                    nc.scalar.activation(
                        out=ypT[:, gs], in_=ppB, func=AF.Identity, bias=0.0
                    )
                    if g >= SCORES_LAG:
                        emit_scores(g - SCORES_LAG)
                    if g >= ZBACK_LAG:
                        gz = g - ZBACK_LAG
                        emit_zback(gz)
                        if gz % 2 == 1:
                            emit_zevac(gz // 2)
                for g in range(ngrp - SCORES_LAG, ngrp):
                    emit_scores(g)
                for g in range(ngrp - ZBACK_LAG, ngrp):
                    emit_zback(g)
                    if g % 2 == 1:
                        emit_zevac(g // 2)
                # rank-2 bias correction, final accumulation into scores
                bs = slice(b * P, (b + 1) * P)
                nc.tensor.matmul(
                    ps_t, l2_sb[:, bs], r2_sb[:, bs], start=False, stop=True
                )
                ctx[b] = {"z": z, "ps": ps_t}

            def emit_softmax(b):
                ps_t = ctx[b]["ps"]
                negmx = sm_pool.tile([P, 1], f32, tag="negmx", name="negmx")
                nc.vector.tensor_reduce(
                    out=negmx, in_=ps_t, axis=AX.X, op=ALU.max, negate=True
                )
                e = sm_pool.tile([P, P], f32, tag="e", name="e")
                se = sm_pool.tile([P, 1], f32, tag="se", name="se")
                nc.scalar.activation(
                    out=e, in_=ps_t, func=AF.Exp, bias=negmx, scale=1.0, accum_out=se
                )
                sse = sm_pool.tile([P, 1], f32, tag="sse", name="sse")
                nc.vector.tensor_scalar_mul(sse, se, sqrt_l)
                rcp = sm_pool.tile([P, 1], f32, tag="rcp", name="rcp")
                nc.vector.reciprocal(rcp, sse)
                attn = sm_pool.tile([P, P], f16, tag="attn", name="attn")
                nc.gpsimd.tensor_scalar_mul(attn, e, rcp)
                pat = psum_sc.tile([P, P], f16, tag="ps", name="pat")
                nc.tensor.transpose(pat, attn, ids)
                attnT = sm_pool.tile([P, P], f16, tag="attnT", name="attnT")
                nc.vector.tensor_copy(out=attnT, in_=pat)
                ctx[b]["attnT"] = attnT

            def emit_out(b):
                attnT = ctx[b]["attnT"]
                z = ctx[b]["z"]
                for oc in range(nout):
                    os_ = slice(oc * OUT_TILE, (oc + 1) * OUT_TILE)
                    po_t = psum_out.tile([P, OUT_TILE], f32, tag="po", name="po_t")
                    nc.tensor.matmul(po_t, attnT, z[:, os_], start=True, stop=True)
                    ot = out_pool.tile([P, OUT_TILE], f16, tag="ot", name="ot")
                    if oc % 2 == 0:
                        nc.vector.tensor_copy(out=ot, in_=po_t)
                    else:
                        nc.scalar.activation(
                            out=ot, in_=po_t, func=AF.Identity, bias=0.0
                        )
                    nc.scalar.dma_start(out_d[b, :, os_], ot)
                del ctx[b]

            for b in range(bpc):
                emit_phase1(b)
                if b > 0:
                    emit_out(b - 1)
                emit_softmax(b)
            emit_out(bpc - 1)

    _split_excess_waits(nc, mybir, max_waits=1)
    return nc


_nc_cache = {}


def _get_nc():
    key = (BPC, L)
    if key not in _nc_cache:
        _nc_cache[key] = build_nc(BPC, L)
    return _nc_cache[key]


def _prep_host(x, y, Wf, bf, Wa, ba):
    """Cast inputs to fp16 and compute the rank-2 scores bias correction."""
    x16 = np.ascontiguousarray(x.astype(np.float16))
    y16 = np.ascontiguousarray(y.astype(np.float16))
    wf16 = Wf.astype(np.float16)
    wa16 = Wa.astype(np.float16)
    # rowsums of the fp16-projected activations (without bias):
    # u0 = Wf @ sum_l x, v0 = Wa @ sum_l y  (fp32 accumulation)
    sx = x16.astype(np.float32).sum(axis=-1)  # (B, 128)
    sy = y16.astype(np.float32).sum(axis=-1)
    u0 = sx @ wf16.astype(np.float32).T  # (B, 128)
    v0 = sy @ wa16.astype(np.float32).T
    nb = x.shape[0]
    l2 = np.empty((nb, 2, P), np.float16)
    r2 = np.empty((nb, 2, P), np.float16)
    l2[:, 0, :] = bf[None, :]
    l2[:, 1, :] = u0
    r2[:, 0, :] = v0 + float(L) * ba[None, :]
    r2[:, 1, :] = ba[None, :]
    wft = np.ascontiguousarray(wf16.T)
    wat = np.ascontiguousarray(wa16.T)
    bzv = (bf + ba).astype(np.float32)
    return x16, y16, wft, wat, bzv, l2, r2


def make_in_maps(x, y, Wf, bf, Wa, ba):
    x = np.asarray(x, dtype=np.float32)
    y = np.asarray(y, dtype=np.float32)
    Wf = np.asarray(Wf, dtype=np.float32)
    bf = np.asarray(bf, dtype=np.float32)
    Wa = np.asarray(Wa, dtype=np.float32)
    ba = np.asarray(ba, dtype=np.float32)
    x16, y16, wft, wat, bzv, l2, r2 = _prep_host(x, y, Wf, bf, Wa, ba)
    in_maps = []
    for c in range(NCORES):
        sl = slice(c * BPC, (c + 1) * BPC)
        in_maps.append(
            {
                "x": np.ascontiguousarray(x16[sl]),
                "y": np.ascontiguousarray(y16[sl]),
                "wft": wft,
                "wat": wat,
                "bzv": bzv,
                "l2": np.ascontiguousarray(l2[sl]),
                "r2": np.ascontiguousarray(r2[sl]),
            }
        )
    return in_maps


def kernel(x, y, Wf, bf, Wa, ba):
    from concourse.bass_utils import run_bass_kernel_spmd

    in_maps = make_in_maps(x, y, Wf, bf, Wa, ba)
    nc = _get_nc()
    res = run_bass_kernel_spmd(nc, in_maps, core_ids=list(range(NCORES)))
    out = np.concatenate([r["out"] for r in res.results], axis=0)
    return np.ascontiguousarray(out.astype(np.float32))


if __name__ == "__main__":
    rng = np.random.default_rng(0)
    inputs = {
        "x": rng.standard_normal((B, P, L), dtype=np.float32),
        "y": rng.standard_normal((B, P, L), dtype=np.float32),
        "Wf": (rng.standard_normal((P, P)) / np.sqrt(P)).astype(np.float32),
        "bf": (rng.standard_normal(P) * 0.02).astype(np.float32),
        "Wa": (rng.standard_normal((P, P)) / np.sqrt(P)).astype(np.float32),
        "ba": (rng.standard_normal(P) * 0.02).astype(np.float32),
    }
    o = kernel(**inputs)
    print(o.shape, o.dtype)
